# revision 1
# baseline (speedup 1.0000x reference)
"""BoundaryLoss Trainium2 kernel.

Computes mean((pred_boundary*w - target_boundary*w)^2) where boundaries are
|conv3d(x, sobel)| of argmax-class / target volumes, w = 3 where target in
SMALL_CLASSES else 1.

Sharding: data-parallel over 8 cores = 2 batches x 4 depth-chunks of 32
slices (+1 halo slice each side, host zero-padded). Each core returns
per-(partition, group) partial sums of 576*(pb-tb)^2*w^2; host does the
mean (the scalar all-reduce across shards).

Math used on-device (exact in fp16/f32 integer arithmetic):
  conv3d(x, K) = (32*x - S_d S_h S_w x) / 24,  S = [1,2,1] separable
  A := 32*pred - S(pred) = 24*pred_boundary_signed   (pred scaled by 32 on dev)
  B := 32*t - S(t) = 24*targ_boundary_signed         (t kept 1x; PSUM holds B/32)
  loss partial = sum( ((|A| - |B|) * w)^2 ) = 576 * sum((pb-tb)^2 w^2)

Argmax trick: key_c = int32bits(v_c + 100.0) << 4 | (15 - c). v+100 in
[90,110) has fixed f32 exponent, so int key order == float order of v, and
low 4 bits give exact first-index tie-breaking. Max tree over 11 keys, then
idx = 15 - (key & 15), pred*32 = 480 - 32*(key & 15).
"""

import numpy as np

B, C, D, H, W = 2, 11, 128, 128, 128
N_CORES = 8
DSH = 32            # output depth slices per shard
DH = DSH + 2        # input slices incl. halo
CHUNKS = (8, 8, 8, 8, 2)
N_GROUPS = DSH // 4  # 8 PSUM groups of 4 output slices

_CACHE = {}


def _group_schedule(chunks, n_groups):
    """groups emitted after each chunk: group g needs input slices <= 4g+5."""
    sched, done = [], 0
    end = 0
    for nd in chunks:
        end += nd
        gs = []
        while done < n_groups and 4 * done + 5 <= end - 1:
            gs.append(done)
            done += 1
        sched.append(gs)
    assert done == n_groups, (sched, done)
    return sched


def _make_wmats():
    """[3,128,128] fp16: identity, -T/32, -T/16 with T = tridiag(1,2,1)."""
    T = np.zeros((128, 128), np.float32)
    i = np.arange(128)
    T[i, i] = 2.0
    T[i[:-1], i[:-1] + 1] = 1.0
    T[i[:-1] + 1, i[:-1]] = 1.0
    wm = np.stack([np.eye(128, dtype=np.float32), -T / 32.0, -T / 16.0])
    return wm.astype(np.float16)


def _build_nc(dh, chunks, debug=False, reps=1, dyn_reps=False, stage="full"):
    # stage: "dma" | "argmax" | "smooth" | "mm" | "full" — prefix subsets for
    # bottleneck isolation (timing experiments only; grading uses "full").
    import concourse.bass as bass  # noqa: F401
    import concourse.bacc as bacc
    import concourse.mybir as mybir
    from concourse.tile import TileContext

    f32, f16, i32 = mybir.dt.float32, mybir.dt.float16, mybir.dt.int32
    A = mybir.AluOpType
    AF = mybir.ActivationFunctionType
    X = mybir.AxisListType.X  # noqa: F841

    dsh = dh - 2
    n_groups = dsh // 4
    sched = _group_schedule(chunks, n_groups)
    max_nd = max(chunks)

    nc = bacc.Bacc()
    lg = nc.declare_dram_parameter("logits", [C, dh, H, W], f32, isOutput=False)
    tg = nc.declare_dram_parameter("target", [dh, H, W], i32, isOutput=False)
    wm = nc.declare_dram_parameter("wmats", [3, 128, 128], f16, isOutput=False)
    out = nc.declare_dram_parameter("out", [128, n_groups], f32, isOutput=True)
    nrp = (nc.declare_dram_parameter("nreps", [1, 1], i32, isOutput=False)
           if dyn_reps else None)
    if debug:
        dbg_pred = nc.declare_dram_parameter("dbg_pred", [128, dh, 130], f32, isOutput=True)
        dbg_wmap = nc.declare_dram_parameter("dbg_wmap", [128, dh, 128], f32, isOutput=True)
        dbg_xswp = nc.declare_dram_parameter("dbg_xswp", [128, dh, 128], f32, isOutput=True)
        dbg_psa = nc.declare_dram_parameter("dbg_psa", [128, n_groups, 512], f32, isOutput=True)
        dbg_psb = nc.declare_dram_parameter("dbg_psb", [128, n_groups, 512], f32, isOutput=True)

    PW = 130  # width padded with a zero column each side

    with TileContext(nc) as tc:
        from contextlib import ExitStack

        with ExitStack() as ctx:
            cpool = ctx.enter_context(tc.tile_pool(name="const", bufs=1))
            lgpool = ctx.enter_context(tc.tile_pool(name="lg", bufs=2 * C))
            tgpool = ctx.enter_context(tc.tile_pool(name="tgt", bufs=2))
            pers = ctx.enter_context(tc.tile_pool(name="pers", bufs=1))
            wkpool = ctx.enter_context(tc.tile_pool(name="wk", bufs=2))
            uvpool = ctx.enter_context(tc.tile_pool(name="uv", bufs=4))
            pspool = ctx.enter_context(tc.tile_pool(name="ps", bufs=4, space="PSUM"))

            # constants
            wt = cpool.tile([128, 3, 128], f16, tag="wt")
            nc.sync.dma_start(out=wt[:, :, :], in_=wm[:, :, :].rearrange("k h m -> h k m"))
            W_I, W_T1, W_T2 = wt[:, 0, :], wt[:, 1, :], wt[:, 2, :]

            # persistent volumes (halo-resident in SBUF)
            ptP = pers.tile([128, dh, PW], f16, tag="ptP")   # 32*pred, w-padded
            ptT = pers.tile([128, dh, PW], f16, tag="ptT")   # target,  w-padded
            xswP = pers.tile([128, dh, 128], f16, tag="xswP")
            xswT = pers.tile([128, dh, 128], f16, tag="xswT")
            wmap = pers.tile([128, dh, 128], f16, tag="wmap")
            acc = pers.tile([128, n_groups], f32, tag="acc")

            # zero whole padded buffers once (on ACT so POOL readers of ptT
            # only ever see ACT history); interior is overwritten per chunk
            nc.scalar.memzero(ptP[:, :, :])
            nc.scalar.memzero(ptT[:, :, :])
            nc.vector.memset(acc[:, :], 0.0)

            def emit_group(g):
                psA = pspool.tile([128, 512], f32, tag="ps")
                psB = pspool.tile([128, 512], f32, tag="ps")
                # identity taps: +32p / +t
                nc.tensor.matmul(psA[:, :], W_I, ptP[:, 4 * g + 1 : 4 * g + 5, 1:129],
                                 start=True, stop=False)
                nc.tensor.matmul(psB[:, :], W_I, ptT[:, 4 * g + 1 : 4 * g + 5, 1:129],
                                 start=True, stop=False)
                # d-1 and d+1 taps (-T/32)
                nc.tensor.matmul(psA[:, :], W_T1, xswP[:, 4 * g : 4 * g + 4, :],
                                 start=False, stop=False)
                nc.tensor.matmul(psA[:, :], W_T1, xswP[:, 4 * g + 2 : 4 * g + 6, :],
                                 start=False, stop=False)
                nc.tensor.matmul(psB[:, :], W_T1, xswT[:, 4 * g : 4 * g + 4, :],
                                 start=False, stop=False)
                nc.tensor.matmul(psB[:, :], W_T1, xswT[:, 4 * g + 2 : 4 * g + 6, :],
                                 start=False, stop=False)
                # d tap (-T/16)
                nc.tensor.matmul(psA[:, :], W_T2, xswP[:, 4 * g + 1 : 4 * g + 5, :],
                                 start=False, stop=True)
                nc.tensor.matmul(psB[:, :], W_T2, xswT[:, 4 * g + 1 : 4 * g + 5, :],
                                 start=False, stop=True)
                # |A|, |B| (B accumulated at 1/32 scale -> scale=32 on abs)
                u = uvpool.tile([128, 512], f16, tag="u")
                v = uvpool.tile([128, 512], f16, tag="v")
                if debug:
                    du = uvpool.tile([128, 512], f32, tag="du")
                    dv = uvpool.tile([128, 512], f32, tag="dv")
                    nc.vector.tensor_copy(du[:, :], psA[:, :])
                    nc.vector.tensor_copy(dv[:, :], psB[:, :])
                    nc.sync.dma_start(
                        out=dbg_psa[:, g, :].rearrange("p w -> p w"), in_=du[:, :])
                    nc.sync.dma_start(
                        out=dbg_psb[:, g, :].rearrange("p w -> p w"), in_=dv[:, :])
                nc.scalar.activation(u[:, :], psA[:, :], AF.Abs)
                nc.scalar.activation(v[:, :], psB[:, :], AF.Abs, scale=32.0)
                e = wkpool.tile([128, 512], f16, tag="e")
                ew = wkpool.tile([128, 512], f16, tag="ew")
                scr = wkpool.tile([128, 512], f32, tag="scr")
                if stage == "mm":
                    nc.scalar.activation(scr[:, :], u[:, :], AF.Square,
                                         accum_out=acc[:, g : g + 1])
                else:
                    nc.vector.tensor_tensor(e[:, :], u[:, :], v[:, :], A.subtract)
                    nc.vector.tensor_tensor(ew[:, :], e[:, :],
                                            wmap[:, 4 * g + 1 : 4 * g + 5, :],
                                            A.mult)
                    nc.scalar.activation(scr[:, :], ew[:, :], AF.Square,
                                         accum_out=acc[:, g : g + 1])

            # optional on-device repeat loop (timing harness only; the acc
            # columns are overwritten, not accumulated, so reps are idempotent)
            if dyn_reps:
                nrt = cpool.tile([1, 1], i32, tag="nrt", name="nrt")
                nc.sync.dma_start(out=nrt[0:1, 0:1], in_=nrp[0:1, 0:1])
                regs = nc.alloc_registers("nreps_r")
                for eng_t, reg in zip(mybir.ALL_ENGINES, regs.handles):
                    nc.engines[eng_t].reg_load(reg, nrt[0:1, 0:1])
                rv = nc.snap(regs, donate=True, min_val=1, max_val=1 << 20)
                rep_cm = tc.For_i(0, rv, 1)
            else:
                rep_cm = tc.For_i(0, reps, 1) if reps > 1 else None
            if rep_cm is not None:
                rep_cm.__enter__()
            d0 = 0
            for ci, nd in enumerate(chunks):
                FD = nd * 128
                # --- DMA logits chunk (11 classes) + target chunk ---
                lts = []
                for c in range(C):
                    t = lgpool.tile([128, max_nd, 128], f32, tag="lg")
                    nc.sync.dma_start(
                        out=t[:, 0:nd, :],
                        in_=lg[c, d0 : d0 + nd, :, :].rearrange("d h w -> h d w"),
                    )
                    lts.append(t)
                tgt = tgpool.tile([128, max_nd, 128], i32, tag="tgt")
                nc.sync.dma_start(
                    out=tgt[:, 0:nd, :],
                    in_=tg[d0 : d0 + nd, :, :].rearrange("d h w -> h d w"),
                )

                def F(t):  # flat f32 view [128, FD]
                    return t[:, 0:nd, :].rearrange("p d w -> p (d w)")

                def I(t):  # flat int32 view
                    return F(t).bitcast(i32)

                if stage == "dma":  # anchor DMAs against DCE, no compute
                    for c in range(C):
                        nc.scalar.activation(acc[0:1, c % 8 : c % 8 + 1],
                                             lts[c][0:1, 0:1, 0:1], AF.Copy)
                    nc.scalar.activation(acc[0:1, 0:1], tgt[0:1, 0:1, 0:1],
                                         AF.Copy)
                    d0 += nd
                    continue

                # --- argmax keys ---
                # POOL instructions cannot wait on DMAHW semaphores (walrus
                # codegen limit), so POOL only ever reads ACT/POOL-written
                # tiles: classes 6-10 keys go to POOL-private kt tiles.
                # k1 = fl(v + 1124.0) (ACT, in-place): the f32 rounding at
                # magnitude ~1124 (ulp 2^-13) quantizes v+100 to a 16-ulp
                # grid, zeroing the low 4 mantissa bits after re-centering.
                for c in range(C):
                    nc.scalar.activation(F(lts[c]), F(lts[c]), AF.Copy,
                                         bias=1124.0)
                # k2 = (k1 - 1024) + (15-c)*2^-17 — exact index packing into
                # the zeroed low bits, pure-arith fused TS (2x-eligible),
                # split DVE (0-5) / POOL (6-10, f32 so Q7 stays on SIMD path)
                for c in range(C):
                    nc.vector.tensor_scalar(F(lts[c]), F(lts[c]), -1024.0,
                                            (15 - c) * 2.0 ** -17, A.add, A.add)
                # max tree, all on DVE (POOL has no TT-max opcode)
                mx = nc.vector.tensor_tensor
                mx(F(lts[0]), F(lts[0]), F(lts[1]), A.max)
                mx(F(lts[2]), F(lts[2]), F(lts[3]), A.max)
                mx(F(lts[4]), F(lts[4]), F(lts[5]), A.max)
                mx(F(lts[6]), F(lts[6]), F(lts[7]), A.max)
                mx(F(lts[8]), F(lts[8]), F(lts[9]), A.max)
                mx(F(lts[0]), F(lts[0]), F(lts[2]), A.max)
                mx(F(lts[4]), F(lts[4]), F(lts[6]), A.max)
                mx(F(lts[8]), F(lts[8]), F(lts[10]), A.max)
                mx(F(lts[0]), F(lts[0]), F(lts[4]), A.max)
                mx(F(lts[0]), F(lts[0]), F(lts[8]), A.max)
                # extract: jt = key & 15; pred*32 = 480 - 32*jt
                nc.vector.tensor_scalar(I(lts[1]), I(lts[0]), 15, None,
                                        A.bitwise_and)
                nc.scalar.activation(ptP[:, d0 : d0 + nd, 1:129],
                                     lts[1][:, 0:nd, :].bitcast(i32),
                                     AF.Copy, scale=-32.0, bias=480.0)
                if stage == "argmax":
                    nc.scalar.activation(acc[0:1, 0:1], ptP[0:1, d0, 1:2],
                                         AF.Copy)
                    nc.scalar.activation(acc[0:1, 1:2], tgt[0:1, 0:1, 0:1],
                                         AF.Copy)
                    d0 += nd
                    continue
                # target cast int32 -> f16
                nc.scalar.activation(ptT[:, d0 : d0 + nd, 1:129],
                                     tgt[:, 0:nd, :], AF.Copy)

                # --- weight map: w = 1 + [t<2] * 2 + [t==4] * 2 ... wait
                # small classes {2,3,5,..,10} get 3; {0,1,4} get 1:
                # w = 3 - 2*([t<2] + [t==4]) -> a=(t<2)*-2, b=(t==4)*-2, w=a+3+b
                # --- weight map on POOL from the raw int32 target (Q7 int
                # SIMD path; f16 on POOL is catastrophically slow) ---
                wa = wkpool.tile([128, max_nd, 128], f32, tag="wa")
                wb = wkpool.tile([128, max_nd, 128], f32, tag="wb")
                wa_f = wa[:, 0:nd, :].rearrange("p d w -> p (d w)")
                wb_f = wb[:, 0:nd, :].rearrange("p d w -> p (d w)")
                tgt_f = tgt[:, 0:nd, :].rearrange("p d w -> p (d w)")
                nc.vector.tensor_scalar(wa_f, tgt_f, 2, -2, A.is_lt, A.mult)
                nc.vector.tensor_scalar(wb_f, tgt_f, 4, -2, A.is_equal, A.mult)
                nc.vector.scalar_tensor_tensor(
                    wmap[:, d0 : d0 + nd, :], wa_f, 3.0, wb_f, A.add, A.add)

                # --- S_w: x = p[w-1] + 2 p[w] + p[w+1] (both volumes DVE f16)
                for pt, xsw in ((ptP, xswP), (ptT, xswT)):
                    nc.vector.scalar_tensor_tensor(
                        xsw[:, d0 : d0 + nd, :], pt[:, d0 : d0 + nd, 1:129], 2.0,
                        pt[:, d0 : d0 + nd, 0:128], A.mult, A.add)
                    nc.vector.tensor_tensor(
                        xsw[:, d0 : d0 + nd, :], xsw[:, d0 : d0 + nd, :],
                        pt[:, d0 : d0 + nd, 2:130], A.add)

                if stage == "smooth":
                    nc.scalar.activation(acc[0:1, 2:3], xswP[0:1, d0, 0:1],
                                         AF.Copy)
                    nc.scalar.activation(acc[0:1, 3:4], xswT[0:1, d0, 0:1],
                                         AF.Copy)
                    nc.scalar.activation(acc[0:1, 4:5], wmap[0:1, d0, 0:1],
                                         AF.Copy)
                else:
                    for g in sched[ci]:
                        emit_group(g)
                d0 += nd
            if rep_cm is not None:
                rep_cm.__exit__(None, None, None)

            if debug:
                dp = pers.tile([128, dh, 130], f32, tag="dp")
                dw = pers.tile([128, dh, 128], f32, tag="dw")
                dx = pers.tile([128, dh, 128], f32, tag="dx")
                nc.vector.tensor_copy(dp[:, :, :], ptP[:, :, :])
                nc.vector.tensor_copy(dw[:, :, :], wmap[:, :, :])
                nc.vector.tensor_copy(dx[:, :, :], xswP[:, :, :])
                nc.sync.dma_start(out=dbg_pred[:, :, :], in_=dp[:, :, :])
                nc.sync.dma_start(out=dbg_wmap[:, :, :], in_=dw[:, :, :])
                nc.sync.dma_start(out=dbg_xswp[:, :, :], in_=dx[:, :, :])
            nc.sync.dma_start(out=out[:, :], in_=acc[:, :])
    nc.compile()
    return nc


def _get_built(dh=DH, chunks=CHUNKS):
    key = (dh, tuple(chunks))
    if key not in _CACHE:
        _CACHE[key] = _build_nc(dh, chunks)
    return _CACHE[key]


def _shard_inputs(logits, target):
    """FULL inputs -> list of 8 per-core in_maps (b-major, then depth chunk)."""
    lp = np.zeros((B, C, D + 2, H, W), np.float32)
    lp[:, :, 1:-1] = logits
    tp = np.zeros((B, 1, D + 2, H, W), np.int32)
    tp[:, :, 1:-1] = target
    wm = _make_wmats()
    maps = []
    for b in range(B):
        for j in range(D // DSH):
            s = j * DSH
            maps.append({
                "logits": np.ascontiguousarray(lp[b, :, s : s + DH]),
                "target": np.ascontiguousarray(tp[b, 0, s : s + DH]),
                "wmats": wm,
            })
    return maps


def kernel(logits: np.ndarray, target: np.ndarray) -> np.ndarray:
    from concourse.bass_utils import run_bass_kernel_spmd

    nc = _get_built()
    maps = _shard_inputs(np.asarray(logits), np.asarray(target))
    res = run_bass_kernel_spmd(nc, maps, list(range(N_CORES))).results
    total = 0.0
    for r in res:
        total += np.asarray(r["out"], np.float64).sum()
    loss = total / (576.0 * B * D * H * W)
    return np.float32(loss)


# ---------------- numpy reference for one shard (testing only) ----------------

def shard_partial_np(lg, tgt):
    """lg [C,dh,H,W] f32 (already +halo, zero-padded), tgt [dh,H,W] i32.
    Returns sum over interior slices of 576*(pb-tb)^2*w^2."""
    pred = np.argmax(lg, axis=0).astype(np.float32)
    t = tgt.astype(np.float32)

    def S(x):
        xp = np.pad(x, ((0, 0), (1, 1), (1, 1)))
        s = xp[:, :, :-2] + 2 * xp[:, :, 1:-1] + xp[:, :, 2:]
        s = s[:, :-2, :] + 2 * s[:, 1:-1, :] + s[:, 2:, :]
        return s[:-2] + 2 * s[1:-1] + s[2:]

    Av = 32 * pred[1:-1] - S(pred)
    Bv = 32 * t[1:-1] - S(t)
    w = np.where((tgt[1:-1] < 2) | (tgt[1:-1] == 4), 1.0, 3.0).astype(np.float32)
    e = (np.abs(Av) - np.abs(Bv)) * w
    return float(np.sum((e * e).astype(np.float64)))



# revision 2
# speedup vs baseline: 1.0011x; 1.0011x over previous
"""BoundaryLoss Trainium2 kernel.

Computes mean((pred_boundary*w - target_boundary*w)^2) where boundaries are
|conv3d(x, sobel)| of argmax-class / target volumes, w = 3 where target in
SMALL_CLASSES else 1.

Sharding: data-parallel over 8 cores = 2 batches x 4 depth-chunks of 32
slices (+1 halo slice each side, host zero-padded). Each core returns
per-(partition, group) partial sums of 576*(pb-tb)^2*w^2; host does the
mean (the scalar all-reduce across shards).

Host-side layout (pure sharding prep): per-core logits are transposed to
[H, C, DH, W] so each per-class chunk DMA is a 2-dim access pattern with
nd*512B contiguous runs per partition (measured 80.6us/rep DMA floor vs
95.5 for the d-major 512B-run layout). target is packed f16 (0..10 exact)
as [H, DH, W] and lands directly in the padded persistent tile.

Device math (exact in f16/f32 integer arithmetic):
  conv3d(x, K) = (32*x - S_d S_h S_w x) / 24,  S = [1,2,1] separable
  A := 32*pred - S(pred) = 24*pred_boundary_signed  (ptP holds 32*pred)
  B := 32*t - S(t)                                  (psB holds B/32)
  loss partial = sum(((|A| - |B|) * w)^2) = 576 * sum((pb-tb)^2 w^2)
S_w runs on DVE f16 as two shifted adds; S_h is the tridiag matmul on PE;
S_d is PSUM accumulation over three d-shifted taps.

Argmax via custom DVE ops (registered in dve_ops at import): key_c =
fl(v_c + 1124) - 1024 + m*2^-17 with m = 10 - c. The f32 rounding at
magnitude ~1124 (ulp 2^-13) quantizes v+100 to a 16-ulp grid, so after
recentering to [90,110) (single binade, ulp 2^-17) the low 4 mantissa
bits hold m exactly; ties resolve to the smallest class like jnp.argmax.
AMAX_PAIRB folds bias+recenter+eps+max into ONE 1x DVE pass per tree
node (epsL = 2*epsR costs no constant slot; classes paired with m ratio
2:1: (10,5),(8,4),(6,3),(2,1)); classes with m=9,7 take an ACT bias pass
+ the 3-constant AMAX_PAIR; m=0 joins via AMAX_TAILB. 10 passes total,
no separate TensorScalar passes. idx = 10 - (key & 15); pred*32 =
320 - 32*(key & 15).

Scheduling: group post-processing (DVE sub/mult + ACT square) is emitted
one chunk late so the in-order DVE never stalls on the PE->ACT PSUM
round-trip; class DMAs issue in tree-consumption order; the out-DMA
issues from the ACT queue so its end-of-rep wait cannot block next-rep
logits DMAs at the head of SP's queue. Pool is deliberately idle: every
attempt to put dependent work on it (Q7 in-order, 95ns launches, slow
sem receive) regressed end-to-end time.

Engine busy per core (cost-model sim): DMA ~75us (the binding roofline),
DVE ~66us, ACT ~40us, PE ~30us. Measured ~92-95us/rep steady-state.
"""

import numpy as np

B, C, D, H, W = 2, 11, 128, 128, 128
N_CORES = 8
DSH = 32            # output depth slices per shard
DH = DSH + 2        # input slices incl. halo
CHUNKS = (8, 8, 8, 6, 4)
N_GROUPS = DSH // 4  # 8 PSUM groups of 4 output slices

_CACHE = {}
_OPS = {}


def _register_custom_ops():
    """Register the fused argmax custom-DVE ops (documented extension point:
    dve_ops.OPS + _SUB_OPCODE_FOR_NAME + CUSTOM_DVE_SPECS). sha pins are
    computed from lower() so they can never drift."""
    global _OPS
    if _OPS:
        return _OPS
    from concourse import dve_ops
    from concourse.dve_spec import Spec, Src0, Src1, C0, C1, C2, maxx, lower
    from concourse.dve_uop import DveOpSpec

    specs = {
        # max((k1L - 1024) + epsL, (k1R - 1024) + epsR) on pre-biased inputs
        "AMAX_PAIR_ANT": Spec(
            body=maxx((Src0 - C0) + C1, (Src1 - C0) + C2),
            reference=lambda in0, in1, s0, s1, imm2: np.maximum(
                (in0 - s0) + s1, (in1 - s0) + np.float32(imm2)),
        ),
        # bias-folded pair: max(fl(v0+1124)-1024 + 2*eps, fl(v1+1124)-1024 + eps)
        "AMAX_PAIRB_ANT": Spec(
            body=maxx(((Src0 + C0) - C1) + (C2 + C2), ((Src1 + C0) - C1) + C2),
            reference=lambda in0, in1, s0, s1, imm2: np.maximum(
                ((in0 + s0) - s1) + np.float32(2 * imm2),
                ((in1 + s0) - s1) + np.float32(imm2)),
        ),
        # max(centered, fl(v+1124)-1024)  (eps = 0 leaf)
        "AMAX_TAILB_ANT": Spec(
            body=maxx(Src0, (Src1 + C0) - C1),
            reference=lambda in0, in1, s0, s1, imm2: np.maximum(
                in0, (in1 + s0) - s1),
        ),
    }
    for name, spec in specs.items():
        if name in dve_ops._SUB_OPCODE_FOR_NAME:
            _OPS[name] = next(o for o in dve_ops.OPS if o.name == name)
            continue
        row = max(dve_ops._SUB_OPCODE_FOR_NAME.values()) + 1
        assert row < 0x20, "custom-DVE row overflow"
        shas = {}
        for ver in ("v3", "v4"):
            try:
                u = lower(spec, ver=ver)
                shas[ver] = DveOpSpec(name=name, opcode=row, uops=u,
                                      rd1_en=True).sha(ver)
            except Exception:
                pass
        op = dve_ops.DveOp(name, spec, subdim=False, uops_sha=shas)
        dve_ops.OPS.append(op)
        dve_ops._SUB_OPCODE_FOR_NAME[name] = row
        dve_ops.CUSTOM_DVE_SPECS[name] = spec
        _OPS[name] = op
    return _OPS


def _group_schedule(chunks, n_groups):
    """groups emitted after each chunk: group g needs input slices <= 4g+5."""
    sched, done = [], 0
    end = 0
    for nd in chunks:
        end += nd
        gs = []
        while done < n_groups and 4 * done + 5 <= end - 1:
            gs.append(done)
            done += 1
        sched.append(gs)
    assert done == n_groups, (sched, done)
    return sched


def _make_wmats():
    """[3,128,128] f16: identity, -T/32, -T/16 with T = tridiag(1,2,1)."""
    T = np.zeros((128, 128), np.float32)
    i = np.arange(128)
    T[i, i] = 2.0
    T[i[:-1], i[:-1] + 1] = 1.0
    T[i[:-1] + 1, i[:-1]] = 1.0
    wm = np.stack([np.eye(128, dtype=np.float32), -T / 32.0, -T / 16.0])
    return wm.astype(np.float16)


EPS = [(10 - c) * 2.0 ** -17 for c in range(C)]


def _build_nc(dh, chunks, reps=1, dyn_reps=False, stage="full", sw_pool=False,
              dma_order=(0, 5, 2, 6, 4, 7, 8, 9, 1, 3, 10), ps_bufs=4):
    # stage: "dma" | "argmax" | "smooth" | "full" — prefix subsets for
    # bottleneck isolation (timing experiments only; grading uses "full").
    import concourse.bacc as bacc
    import concourse.mybir as mybir
    from concourse.tile import TileContext

    ops = _register_custom_ops()
    # S_w smoothing engine: DVE (f16 2x) or Pool (frees DVE, in-order risk)
    PAIR = ops["AMAX_PAIR_ANT"]
    PAIRB, TAILB = ops["AMAX_PAIRB_ANT"], ops["AMAX_TAILB_ANT"]

    f32, f16, i32 = mybir.dt.float32, mybir.dt.float16, mybir.dt.int32
    A = mybir.AluOpType
    AF = mybir.ActivationFunctionType

    dsh = dh - 2
    n_groups = dsh // 4
    sched = _group_schedule(chunks, n_groups)
    max_nd = max(chunks)

    nc = bacc.Bacc()
    SW_ENG = nc.gpsimd if sw_pool else nc.vector
    lg = nc.declare_dram_parameter("logits", [H, C, dh, W], f32, isOutput=False)
    tg = nc.declare_dram_parameter("target", [H, dh, W], f16, isOutput=False)
    wm = nc.declare_dram_parameter("wmats", [3, 128, 128], f16, isOutput=False)
    out = nc.declare_dram_parameter("out", [128, n_groups], f32, isOutput=True)
    nrp = (nc.declare_dram_parameter("nreps", [1, 1], i32, isOutput=False)
           if dyn_reps else None)

    PW = 130  # width padded with a zero column each side

    with TileContext(nc) as tc:
        from contextlib import ExitStack

        with ExitStack() as ctx:
            cpool = ctx.enter_context(tc.tile_pool(name="const", bufs=1))
            lgpool = ctx.enter_context(tc.tile_pool(name="lg", bufs=2 * C))
            pers = ctx.enter_context(tc.tile_pool(name="pers", bufs=1))
            swpool = ctx.enter_context(tc.tile_pool(name="sw", bufs=4))
            wkpool = ctx.enter_context(tc.tile_pool(name="wk", bufs=4))
            uvpool = ctx.enter_context(tc.tile_pool(name="uv", bufs=6))
            pspool = ctx.enter_context(tc.tile_pool(name="ps", bufs=ps_bufs, space="PSUM"))

            # constants
            wt = cpool.tile([128, 3, 128], f16, tag="wt")
            nc.sync.dma_start(out=wt[:, :, :], in_=wm[:, :, :].rearrange("k h m -> h k m"))
            W_I, W_T1, W_T2 = wt[:, 0, :], wt[:, 1, :], wt[:, 2, :]

            # persistent volumes (halo-resident in SBUF)
            ptP = pers.tile([128, dh, PW], f16, tag="ptP")   # 32*pred, w-padded
            ptT = pers.tile([128, dh, PW], f16, tag="ptT")   # target,  w-padded
            xswP = pers.tile([128, dh, 128], f16, tag="xswP")
            xswT = pers.tile([128, dh, 128], f16, tag="xswT")
            wmap = pers.tile([128, dh, 128], f16, tag="wmap")
            acc = pers.tile([128, n_groups], f32, tag="acc")

            # zero padded buffers once; interiors rewritten per chunk
            nc.scalar.memzero(ptP[:, :, :])
            nc.scalar.memzero(ptT[:, :, :])
            nc.vector.memset(acc[:, :], 0.0)

            uv_of = {}

            def emit_group_mm(g):
                """matmul taps + PSUM-draining abs; all psA taps first so
                abs(psA) overlaps the psB matmuls."""
                psA = pspool.tile([128, 512], f32, tag="ps")
                psB = pspool.tile([128, 512], f32, tag="ps")
                nc.tensor.matmul(psA[:, :], W_I, ptP[:, 4 * g + 1 : 4 * g + 5, 1:129],
                                 start=True, stop=False)
                nc.tensor.matmul(psA[:, :], W_T1, xswP[:, 4 * g : 4 * g + 4, :],
                                 start=False, stop=False)
                nc.tensor.matmul(psA[:, :], W_T1, xswP[:, 4 * g + 2 : 4 * g + 6, :],
                                 start=False, stop=False)
                nc.tensor.matmul(psA[:, :], W_T2, xswP[:, 4 * g + 1 : 4 * g + 5, :],
                                 start=False, stop=True)
                nc.tensor.matmul(psB[:, :], W_I, ptT[:, 4 * g + 1 : 4 * g + 5, 1:129],
                                 start=True, stop=False)
                nc.tensor.matmul(psB[:, :], W_T1, xswT[:, 4 * g : 4 * g + 4, :],
                                 start=False, stop=False)
                nc.tensor.matmul(psB[:, :], W_T1, xswT[:, 4 * g + 2 : 4 * g + 6, :],
                                 start=False, stop=False)
                nc.tensor.matmul(psB[:, :], W_T2, xswT[:, 4 * g + 1 : 4 * g + 5, :],
                                 start=False, stop=True)
                # |A|, |B| (B accumulated at 1/32 scale -> scale=32 on abs)
                u = uvpool.tile([128, 512], f16, tag="u")
                v = uvpool.tile([128, 512], f16, tag="v")
                nc.scalar.activation(u[:, :], psA[:, :], AF.Abs)
                nc.scalar.activation(v[:, :], psB[:, :], AF.Abs, scale=32.0)
                uv_of[g] = (u, v)

            def emit_group_post(g):
                """DVE sub/mult + ACT square, emitted one chunk late so the
                in-order DVE never stalls on the PE->ACT round-trip."""
                u, v = uv_of.pop(g)
                e = wkpool.tile([128, 512], f16, tag="e")
                ew = wkpool.tile([128, 512], f16, tag="ew")
                scr = wkpool.tile([128, 512], f32, tag="scr")
                nc.vector.tensor_tensor(e[:, :], u[:, :], v[:, :], A.subtract)
                nc.vector.tensor_tensor(ew[:, :], e[:, :],
                                        wmap[:, 4 * g + 1 : 4 * g + 5, :],
                                        A.mult)
                nc.scalar.activation(scr[:, :], ew[:, :], AF.Square,
                                     accum_out=acc[:, g : g + 1])

            if dyn_reps:
                nrt = cpool.tile([1, 1], i32, tag="nrt", name="nrt")
                nc.sync.dma_start(out=nrt[0:1, 0:1], in_=nrp[0:1, 0:1])
                regs = nc.alloc_registers("nreps_r")
                for eng_t, reg in zip(mybir.ALL_ENGINES, regs.handles):
                    nc.engines[eng_t].reg_load(reg, nrt[0:1, 0:1])
                rv = nc.snap(regs, donate=True, min_val=1, max_val=1 << 20)
                rep_cm = tc.For_i(0, rv, 1)
            else:
                rep_cm = tc.For_i(0, reps, 1) if reps > 1 else None
            if rep_cm is not None:
                rep_cm.__enter__()
            d0 = 0
            for ci, nd in enumerate(chunks):
                FD = nd * 128
                # class DMAs in tree-consumption order: each PAIRB can fire
                # as soon as its two operand classes have landed
                lts = [None] * C
                for c in dma_order:
                    t = lgpool.tile([128, max_nd * 128], f32, tag="lg",
                                    name=f"lg{c}")
                    nc.sync.dma_start(
                        out=t[:, 0 : nd * 128],
                        in_=lg[:, c, d0 : d0 + nd, :].rearrange("h d w -> h (d w)"),
                    )
                    lts[c] = t
                # target arrives as f16 (host-packed; 0..10 exact) straight
                # into the padded persistent tile; its smoothing rides on
                # Pool (DMA-gated only, so Pool never blocks behind compute)
                nc.sync.dma_start(out=ptT[:, d0 : d0 + nd, 1:129],
                                  in_=tg[:, d0 : d0 + nd, :])
                if stage != "dma":
                    t1T = swpool.tile([128, max_nd, 129], f16, tag="t1T")
                    nc.vector.tensor_tensor(t1T[:, 0:nd, :],
                                            ptT[:, d0 : d0 + nd, 0:129],
                                            ptT[:, d0 : d0 + nd, 1:130], A.add)
                    nc.vector.tensor_tensor(xswT[:, d0 : d0 + nd, :],
                                            t1T[:, 0:nd, 0:128],
                                            t1T[:, 0:nd, 1:129], A.add)

                def F(t):  # flat f32 view [128, FD]
                    return t[:, 0:FD]

                def V(t):  # [128, nd, 128] view
                    return t[:, 0:FD].rearrange("p (d w) -> p d w", w=128)

                def I(t):  # flat int32 view
                    return F(t).bitcast(i32)

                if stage == "dma":
                    for c in range(C):
                        nc.scalar.activation(acc[0:1, c % 8 : c % 8 + 1],
                                             lts[c][0:1, 0:1], AF.Copy)
                    nc.scalar.activation(acc[0:1, 0:1], ptT[0:1, d0, 1:2],
                                         AF.Copy)
                    d0 += nd
                    continue

                # --- argmax: fused custom max tree (DVE), bias folded in ---
                # k = fl(v + 1124) - 1024 + m*2^-17, m = 10 - c: the f32
                # rounding at magnitude ~1124 (ulp 2^-13) quantizes v+100 to a
                # 16-ulp grid, so the low 4 mantissa bits carry m exactly.
                # PAIRB folds the bias and pairs classes with m ratio 2:1
                # (epsL = 2*imm2 costs no constant slot). Classes 1,3 (m=9,7)
                # go through an ACT bias + the 3-constant PAIR op.
                E = 2.0 ** -17
                for c in (1, 3):
                    nc.scalar.activation(F(lts[c]), F(lts[c]), AF.Copy,
                                         bias=1124.0)
                for a, b, mr in ((0, 5, 5), (2, 6, 4), (4, 7, 3), (8, 9, 1)):
                    nc.vector._custom_dve(PAIRB, out=F(lts[a]), in0=F(lts[a]),
                                          in1=F(lts[b]), s0=1124.0, s1=1024.0,
                                          imm2=mr * E)
                nc.vector._custom_dve(PAIR, out=F(lts[1]), in0=F(lts[1]),
                                      in1=F(lts[3]), s0=1024.0,
                                      s1=9 * E, imm2=7 * E)
                mx = nc.vector.tensor_tensor
                mx(F(lts[0]), F(lts[0]), F(lts[2]), A.max)
                mx(F(lts[4]), F(lts[4]), F(lts[8]), A.max)
                nc.vector._custom_dve(TAILB, out=F(lts[1]), in0=F(lts[1]),
                                      in1=F(lts[10]), s0=1124.0, s1=1024.0)
                mx(F(lts[0]), F(lts[0]), F(lts[4]), A.max)
                mx(F(lts[0]), F(lts[0]), F(lts[1]), A.max)
                # extract: jt = key & 15; pred*32 = 320 - 32*jt
                nc.vector.tensor_scalar(I(lts[1]), I(lts[0]), 15, None,
                                        A.bitwise_and)
                nc.scalar.activation(ptP[:, d0 : d0 + nd, 1:129],
                                     V(lts[1]).bitcast(i32),
                                     AF.Copy, scale=-32.0, bias=320.0)
                if stage == "argmax":
                    nc.scalar.activation(acc[0:1, 0:1], ptP[0:1, d0, 1:2],
                                         AF.Copy)
                    d0 += nd
                    continue

                # --- pred-side smoothing ---
                t1P = swpool.tile([128, max_nd, 129], f16, tag="t1P")
                SW_ENG.tensor_tensor(t1P[:, 0:nd, :], ptP[:, d0 : d0 + nd, 0:129],
                                        ptP[:, d0 : d0 + nd, 1:130], A.add)
                SW_ENG.tensor_tensor(xswP[:, d0 : d0 + nd, :],
                                        t1P[:, 0:nd, 0:128], t1P[:, 0:nd, 1:129],
                                        A.add)

                # --- weight map: w = 3 - 2*[t<2] - 2*[t==4] (DVE f16 + ACT +3)
                wa = wkpool.tile([128, max_nd, 128], f16, tag="wa")
                wb = wkpool.tile([128, max_nd, 128], f16, tag="wb")
                nc.vector.tensor_scalar(wa[:, 0:nd, :], ptT[:, d0 : d0 + nd, 1:129],
                                        2.0, -2.0, A.is_lt, A.mult)
                nc.vector.tensor_scalar(wb[:, 0:nd, :], ptT[:, d0 : d0 + nd, 1:129],
                                        4.0, -2.0, A.is_equal, A.mult)
                nc.vector.tensor_tensor(wmap[:, d0 : d0 + nd, :], wa[:, 0:nd, :],
                                        wb[:, 0:nd, :], A.add)
                nc.scalar.activation(wmap[:, d0 : d0 + nd, :],
                                     wmap[:, d0 : d0 + nd, :], AF.Copy, bias=3.0)

                if stage == "smooth":
                    nc.scalar.activation(acc[0:1, 2:3], xswP[0:1, d0, 0:1],
                                         AF.Copy)
                    nc.scalar.activation(acc[0:1, 3:4], xswT[0:1, d0, 0:1],
                                         AF.Copy)
                    nc.scalar.activation(acc[0:1, 4:5], wmap[0:1, d0, 0:1],
                                         AF.Copy)
                else:
                    for g in sched[ci]:
                        emit_group_mm(g)
                    if ci > 0:
                        for g in sched[ci - 1]:
                            emit_group_post(g)
                d0 += nd
            if stage == "full":
                for g in sched[len(chunks) - 1]:
                    emit_group_post(g)
            if rep_cm is not None:
                rep_cm.__exit__(None, None, None)

            # out-DMA from the ACT queue: its end-of-rep wait must not sit
            # at the head of SP's queue blocking next-rep logits DMA issues
            nc.scalar.dma_start(out=out[:, :], in_=acc[:, :])
    nc.compile()
    return nc


def _get_built(dh=DH, chunks=CHUNKS):
    key = (dh, tuple(chunks))
    if key not in _CACHE:
        _CACHE[key] = _build_nc(dh, chunks)
    return _CACHE[key]


def _shard_inputs(logits, target):
    """FULL inputs -> 8 per-core in_maps (b-major, then depth chunk).
    Host work is sharding + layout transposes only."""
    lp = np.zeros((B, C, D + 2, H, W), np.float32)
    lp[:, :, 1:-1] = logits
    tp = np.zeros((B, D + 2, H, W), np.float16)
    tp[:, 1:-1] = target[:, 0]  # 0..10, exact in f16
    wm = _make_wmats()
    maps = []
    for b in range(B):
        for j in range(D // DSH):
            s = j * DSH
            maps.append({
                "logits": np.ascontiguousarray(
                    lp[b, :, s : s + DH].transpose(2, 0, 1, 3)),   # [H,C,DH,W]
                "target": np.ascontiguousarray(
                    tp[b, s : s + DH].transpose(1, 0, 2)),         # [H,DH,W]
                "wmats": wm,
            })
    return maps


def kernel(logits: np.ndarray, target: np.ndarray) -> np.ndarray:
    from concourse.bass_utils import run_bass_kernel_spmd

    nc = _get_built()
    maps = _shard_inputs(np.asarray(logits), np.asarray(target))
    res = run_bass_kernel_spmd(nc, maps, list(range(N_CORES))).results
    total = 0.0
    for r in res:
        total += np.asarray(r["out"], np.float64).sum()
    loss = total / (576.0 * B * D * H * W)
    return np.float32(loss)


# ---------------- numpy reference for one shard (testing only) ----------------

def shard_partial_np(lg, tgt):
    """lg [C,dh,H,W] f32 (already +halo, zero-padded), tgt [dh,H,W] i32.
    Returns sum over interior slices of 576*(pb-tb)^2*w^2."""
    pred = np.argmax(lg, axis=0).astype(np.float32)
    t = tgt.astype(np.float32)

    def S(x):
        xp = np.pad(x, ((0, 0), (1, 1), (1, 1)))
        s = xp[:, :, :-2] + 2 * xp[:, :, 1:-1] + xp[:, :, 2:]
        s = s[:, :-2, :] + 2 * s[:, 1:-1, :] + s[:, 2:, :]
        return s[:-2] + 2 * s[1:-1] + s[2:]

    Av = 32 * pred[1:-1] - S(pred)
    Bv = 32 * t[1:-1] - S(t)
    w = np.where((tgt[1:-1] < 2) | (tgt[1:-1] == 4), 1.0, 3.0).astype(np.float32)
    e = (np.abs(Av) - np.abs(Bv)) * w
    return float(np.sum((e * e).astype(np.float64)))
